# revision 33
# baseline (speedup 1.0000x reference)
"""Trainium2 Bass kernel for nn_Ir_Consistency_Loss (gnn_message_passing).

loss = mean_e (1 - re[src_e].re[dst_e]) * ||ir_h[src_e] - ir_h[dst_e]||^2

Pure-streaming, edge-parallel design across 8 NeuronCores, transposed
(feature-dim-on-partitions) layout, fp8_e4m3 stream dtype.

The host pre-gathers per-edge node rows for BOTH endpoints into one
fp8 tensor per core (feature dim on partitions), laid out
[d, tile, j, e] so each partition reads 16 KB contiguous per tile:

  j = 0:re[src] 1:re[dst] 2:ir[src] 3:ir[dst]

Engine facts measured on HW:
  - DVE tensor_tensor runs 1x on fp8 (~1.04 ns/elem/lane).
  - Pool (gpsimd) elementwise shares SBUF ports with DVE; offloading
    there is a net LOSS. Pool does nothing here.
  - PE can compute u - v into PSUM via two accumulated identity
    matmuls (lhsT=I then lhsT=-I), offloading elementwise subtracts.
  - ACT activation is 1x, dtype-independent, (N+352)/1.2 ns, and can
    read PSUM directly.

Per 4096-edge tile (one 2.1 MB DMA):
  - DVE:  w_r = u_r * v_r (fp8->bf16, all 4096 cols)
          diffV = u_h - v_h on cols [PECOL:4096]
  - PE :  diff on cols [0:PECOL) as 512-col PSUM banks:
          psD = I @ u_cols - I @ v_cols  (fp8 identity weights; one
          bank = one accumulation region, pairs kept adjacent)
  - ACT:  sq[:, batch] = Square(psD) per batch; sq[:, 2560:] =
          Square(diffV); all into one sq tile
  - PE :  (emitted next iteration to avoid in-order stalls) per
          128-edge chunk c, ones-matmuls reduce over the partition
          axis: psA[:,c] = agree_e, psB[:,c] = sqsum_e
  - ACT:  stages psB into SBUF (DVE has a single PSUM read port)
  - DVE:  scalar_tensor_tensor (agree-1)*sqsum -> per-tile partials
          (emitted 2 iterations late, same reason)
  - Pad edges are all-zero rows: (0 - 1) * 0 = 0 contribution.
  - Host: loss = -(sum of per-core partials) / E.
"""

import numpy as np
import ml_dtypes

import concourse.bacc as bacc
import concourse.bass as bass
import concourse.mybir as mybir
import concourse.tile as tile
from concourse.bass_utils import run_bass_kernel_spmd

N_NODES = 50000
N_EDGES = 1600000
D = 128
N_CORES = 8
P = 128
CHUNK = 32                 # 128-edge chunks per tile
TILE_E = P * CHUNK         # 4096 edges per tile
EPC = N_EDGES // N_CORES   # 200000 edges per core
T = -(-EPC // TILE_E)      # 49 tiles per core
PAD_E = T * TILE_E         # 200704 padded edges per core
PEC = 28                   # diff chunks computed on PE (cols [0:PEC*128))
BB = 4                     # chunks per PSUM diff batch (512 cols = 1 bank)
NBAT = PEC // BB           # PSUM diff batches per tile
PECOL = PEC * P            # 2560

_cache = {}


def _build_program():
    if "nc" in _cache:
        return _cache["nc"]
    nc = bacc.Bacc("TRN2", target_bir_lowering=False, debug=False,
                   num_devices=N_CORES)
    f8 = mybir.dt.float8e4
    bf16 = mybir.dt.bfloat16
    fp32 = mybir.dt.float32
    # [P, T, 4, TILE_E]: each partition reads 16 KB contiguous per tile
    in4 = nc.dram_tensor("in4", [P, T * 4 * TILE_E], f8,
                         kind="ExternalInput")
    eye = nc.dram_tensor("eye", [2 * P, P], f8, kind="ExternalInput")
    out = nc.dram_tensor("partial", [P, 1], fp32, kind="ExternalOutput")

    Alu = mybir.AluOpType
    X = mybir.AxisListType.X
    Sq = mybir.ActivationFunctionType.Square
    Cp = mybir.ActivationFunctionType.Copy

    LAG = 2  # iterations of lag for the combine stage

    with tile.TileContext(nc) as tc:
        with (
            tc.tile_pool(name="in", bufs=4) as ipool,
            tc.tile_pool(name="work", bufs=3) as wpool,
            tc.tile_pool(name="psd", bufs=5, space="PSUM") as pdpool,
            tc.tile_pool(name="ps", bufs=3, space="PSUM") as pspool,
            tc.tile_pool(name="cmb", bufs=LAG + 2) as cpool,
            tc.tile_pool(name="stats", bufs=1) as stpool,
        ):
            partials = stpool.tile([P, T], fp32, tag="partials")
            ones = stpool.tile([P, 1], bf16, tag="ones")
            nc.vector.memset(ones[:], 1.0)
            eyeP = stpool.tile([P, P], f8, tag="eyeP")
            eyeN = stpool.tile([P, P], f8, tag="eyeN")
            nc.sync.dma_start(out=eyeP[:], in_=eye[0:P, :])
            nc.sync.dma_start(out=eyeN[:], in_=eye[P:2 * P, :])

            pend_red = {}
            pend_cmb = {}

            def reduce_stage(t):
                # ones-matmul reductions for tile t (emitted during
                # iteration t+1 so PE never waits on ACT mid-queue)
                w_r, sq = pend_red.pop(t)
                psAB = pspool.tile([P, 2, CHUNK], fp32, tag="psAB")
                psA = psAB[:, 0, :]
                psB = psAB[:, 1, :]
                for c in range(CHUNK):
                    nc.tensor.matmul(psA[:, c:c + 1],
                                     w_r[:, c * P:(c + 1) * P], ones[:],
                                     start=True, stop=True)
                    nc.tensor.matmul(psB[:, c:c + 1],
                                     sq[:, c * P:(c + 1) * P], ones[:],
                                     start=True, stop=True)
                pend_cmb[t] = psAB

            def combine(t):
                psAB = pend_cmb.pop(t)
                psA = psAB[:, 0, :]
                psB = psAB[:, 1, :]
                bS = cpool.tile([P, CHUNK], fp32, tag="bS")
                junk = cpool.tile([P, CHUNK], fp32, tag="junk")
                # DVE may read at most one PSUM operand per instruction
                # (single PSUM read port) -> stage psB into SBUF first.
                # On DVE (tiny copy) since ACT is nearer its budget here.
                nc.vector.tensor_copy(bS[:], psB[:])
                nc.vector.scalar_tensor_tensor(
                    out=junk[:], in0=psA[:], scalar=1.0, in1=bS[:],
                    op0=Alu.subtract, op1=Alu.mult,
                    accum_out=partials[:, t:t + 1])

            for t in range(T):
                s4 = ipool.tile([P, 4, TILE_E], f8, tag="s4")
                in_ap = bass.AP(tensor=in4[:].tensor,
                                offset=t * 4 * TILE_E,
                                ap=[[T * 4 * TILE_E, P], [TILE_E, 4],
                                    [1, TILE_E]])
                nc.sync.dma_start(out=s4[:], in_=in_ap)

                w_r = wpool.tile([P, TILE_E], bf16, tag="w_r")
                diffV = wpool.tile([P, TILE_E - PECOL], bf16, tag="diffV")
                sq = wpool.tile([P, TILE_E], bf16, tag="sq")

                nc.vector.tensor_tensor(out=w_r[:], in0=s4[:, 0, :],
                                        in1=s4[:, 1, :], op=Alu.mult)
                nc.vector.tensor_tensor(out=diffV[:],
                                        in0=s4[:, 2, PECOL:],
                                        in1=s4[:, 3, PECOL:],
                                        op=Alu.subtract)

                # one 512-col PSUM bank = one accumulation region; a
                # single adjacent (I @ u, then -I @ v accumulate) pair
                # per bank avoids intra-bank group interleaving, which
                # corrupts PSUM accumulation.
                for b in range(NBAT):
                    psD = pdpool.tile([P, BB * P], fp32, tag="psD")
                    c0 = b * BB * P
                    nc.tensor.matmul(psD[:], eyeP[:],
                                     s4[:, 2, c0:c0 + BB * P],
                                     start=True, stop=False)
                    nc.tensor.matmul(psD[:], eyeN[:],
                                     s4[:, 3, c0:c0 + BB * P],
                                     start=False, stop=True)
                    nc.scalar.activation(out=sq[:, c0:c0 + BB * P],
                                         in_=psD[:], func=Sq)
                nc.scalar.activation(out=sq[:, PECOL:], in_=diffV[:],
                                     func=Sq)

                pend_red[t] = (w_r, sq)
                if t >= 1:
                    reduce_stage(t - 1)
                if t >= LAG:
                    combine(t - LAG)

            reduce_stage(T - 1)
            for t in range(max(0, T - LAG), T):
                combine(t)

            total = stpool.tile([P, 1], fp32, tag="total")
            nc.vector.tensor_reduce(out=total[:], in_=partials[:], axis=X,
                                    op=Alu.add)
            nc.sync.dma_start(out=out[:], in_=total[:])
    nc.compile()
    _cache["nc"] = nc
    return nc


def kernel(re_, ir_h, src, dst):
    re_ = np.asarray(re_, dtype=np.float32)
    ir_h = np.asarray(ir_h, dtype=np.float32)
    g2r = np.ascontiguousarray(
        re_.T.astype(ml_dtypes.float8_e4m3))        # [128, N]
    g2h = np.ascontiguousarray(
        ir_h.T.astype(ml_dtypes.float8_e4m3))       # [128, N]
    eye = np.concatenate([np.eye(P, dtype=np.float32),
                          -np.eye(P, dtype=np.float32)]
                         ).astype(ml_dtypes.float8_e4m3)

    s = np.asarray(src).astype(np.int64)
    d = np.asarray(dst).astype(np.int64)
    e_total = s.shape[0]

    in_maps = []
    for c in range(N_CORES):
        lo, hi = c * EPC, (c + 1) * EPC
        arr = np.zeros((4, P, PAD_E), ml_dtypes.float8_e4m3)
        arr[0, :, :EPC] = g2r[:, s[lo:hi]]
        arr[1, :, :EPC] = g2r[:, d[lo:hi]]
        arr[2, :, :EPC] = g2h[:, s[lo:hi]]
        arr[3, :, :EPC] = g2h[:, d[lo:hi]]
        # [4, P, T, TILE_E] -> [P, T, 4, TILE_E] (16 KB contiguous per
        # partition per tile)
        a4 = np.ascontiguousarray(
            arr.reshape(4, P, T, TILE_E).transpose(1, 2, 0, 3))
        in_maps.append({"in4": a4.reshape(P, T * 4 * TILE_E), "eye": eye})

    nc = _build_program()
    try:
        res = run_bass_kernel_spmd(nc, in_maps,
                                   core_ids=list(range(N_CORES)))
    except Exception:
        # transient NRT device wedge -- one retry is usually enough
        res = run_bass_kernel_spmd(nc, in_maps,
                                   core_ids=list(range(N_CORES)))
    tot = 0.0
    for r in res.results:
        tot += float(r["partial"].sum(dtype=np.float64))
    return np.float32(-tot / e_total)


# revision 34
# speedup vs baseline: 1.3163x; 1.3163x over previous
"""Trainium2 Bass kernel for nn_Ir_Consistency_Loss (gnn_message_passing).

loss = mean_e (1 - re[src_e].re[dst_e]) * ||ir_h[src_e] - ir_h[dst_e]||^2

Pure-streaming, edge-parallel design across 8 NeuronCores, transposed
(feature-dim-on-partitions) layout, fp8_e4m3 stream dtype.

The host pre-gathers per-edge node rows for BOTH endpoints into one
fp8 tensor per core (feature dim on partitions), laid out
[d, tile, j, e] so each partition reads 16 KB contiguous per tile:

  j = 0:re[src] 1:re[dst] 2:ir[src] 3:ir[dst]

Engine facts measured on HW:
  - DVE tensor_tensor runs 1x on fp8 (~1.04 ns/elem/lane).
  - Pool (gpsimd) elementwise shares SBUF ports with DVE; offloading
    there is a net LOSS. Pool does nothing here.
  - PE can compute u - v into PSUM via two accumulated identity
    matmuls (lhsT=I then lhsT=-I), offloading elementwise subtracts.
  - ACT activation is 1x, dtype-independent, (N+352)/1.2 ns, and can
    read PSUM directly.

Per 4096-edge tile (one 2.1 MB DMA):
  - DVE:  w_r = u_r * v_r (fp8->bf16, all 4096 cols)
          diffV = u_h - v_h on cols [PECOL:4096]
  - PE :  diff on cols [0:PECOL) as 512-col PSUM banks:
          psD = I @ u_cols - I @ v_cols  (fp8 identity weights; one
          bank = one accumulation region, pairs kept adjacent)
  - ACT:  sq[:, batch] = Square(psD) per batch; sq[:, 2560:] =
          Square(diffV); all into one sq tile
  - PE :  (emitted next iteration to avoid in-order stalls) per
          128-edge chunk c, ones-matmuls reduce over the partition
          axis: psA[:,c] = agree_e, psB[:,c] = sqsum_e
  - ACT:  stages psB into SBUF (DVE has a single PSUM read port)
  - DVE:  scalar_tensor_tensor (agree-1)*sqsum -> per-tile partials
          (emitted 2 iterations late, same reason)
  - Pad edges are all-zero rows: (0 - 1) * 0 = 0 contribution.
  - Host: loss = -(sum of per-core partials) / E.
"""

import numpy as np
import ml_dtypes

import concourse.bacc as bacc
import concourse.bass as bass
import concourse.mybir as mybir
import concourse.tile as tile
from concourse.bass_utils import run_bass_kernel_spmd

N_NODES = 50000
N_EDGES = 1600000
D = 128
N_CORES = 8
P = 128
CHUNK = 32                 # 128-edge chunks per tile
TILE_E = P * CHUNK         # 4096 edges per tile
EPC = N_EDGES // N_CORES   # 200000 edges per core
T = -(-EPC // TILE_E)      # 49 tiles per core
PAD_E = T * TILE_E         # 200704 padded edges per core
PEC = 24                   # diff chunks computed on PE (cols [0:PEC*128))
BB = 4                     # chunks per PSUM diff batch (512 cols = 1 bank)
NBAT = PEC // BB           # PSUM diff batches per tile
PECOL = PEC * P            # 2560

_cache = {}


def _build_program():
    if "nc" in _cache:
        return _cache["nc"]
    nc = bacc.Bacc("TRN2", target_bir_lowering=False, debug=False,
                   num_devices=N_CORES)
    f8 = mybir.dt.float8e4
    bf16 = mybir.dt.bfloat16
    fp32 = mybir.dt.float32
    # [P, T, 4, TILE_E]: each partition reads 16 KB contiguous per tile
    in4 = nc.dram_tensor("in4", [P, T * 4 * TILE_E], f8,
                         kind="ExternalInput")
    eye = nc.dram_tensor("eye", [2 * P, P], f8, kind="ExternalInput")
    out = nc.dram_tensor("partial", [P, 1], fp32, kind="ExternalOutput")

    Alu = mybir.AluOpType
    X = mybir.AxisListType.X
    Sq = mybir.ActivationFunctionType.Square
    Cp = mybir.ActivationFunctionType.Copy

    LAG = 2  # iterations of lag for the combine stage

    with tile.TileContext(nc) as tc:
        with (
            tc.tile_pool(name="in", bufs=4) as ipool,
            tc.tile_pool(name="work", bufs=3) as wpool,
            tc.tile_pool(name="psd", bufs=3, space="PSUM") as pdpool,
            tc.tile_pool(name="ps", bufs=3, space="PSUM") as pspool,
            tc.tile_pool(name="cmb", bufs=LAG + 2) as cpool,
            tc.tile_pool(name="stats", bufs=1) as stpool,
        ):
            partials = stpool.tile([P, T], fp32, tag="partials")
            ones = stpool.tile([P, 1], bf16, tag="ones")
            nc.vector.memset(ones[:], 1.0)
            eyeP = stpool.tile([P, P], f8, tag="eyeP")
            eyeN = stpool.tile([P, P], f8, tag="eyeN")
            nc.sync.dma_start(out=eyeP[:], in_=eye[0:P, :])
            nc.sync.dma_start(out=eyeN[:], in_=eye[P:2 * P, :])

            pend_red = {}
            pend_cmb = {}

            def reduce_stage(t):
                # ones-matmul reductions for tile t (emitted during
                # iteration t+1 so PE never waits on ACT mid-queue)
                w_r, sq = pend_red.pop(t)
                psAB = pspool.tile([P, 2, CHUNK], fp32, tag="psAB")
                psA = psAB[:, 0, :]
                psB = psAB[:, 1, :]
                for c in range(CHUNK):
                    nc.tensor.matmul(psA[:, c:c + 1],
                                     w_r[:, c * P:(c + 1) * P], ones[:],
                                     start=True, stop=True)
                    nc.tensor.matmul(psB[:, c:c + 1],
                                     sq[:, c * P:(c + 1) * P], ones[:],
                                     start=True, stop=True)
                pend_cmb[t] = psAB

            def combine(t):
                psAB = pend_cmb.pop(t)
                psA = psAB[:, 0, :]
                psB = psAB[:, 1, :]
                bS = cpool.tile([P, CHUNK], fp32, tag="bS")
                junk = cpool.tile([P, CHUNK], fp32, tag="junk")
                # DVE may read at most one PSUM operand per instruction
                # (single PSUM read port) -> ACT stages psB into SBUF.
                nc.scalar.activation(out=bS[:], in_=psB[:], func=Cp)
                nc.vector.scalar_tensor_tensor(
                    out=junk[:], in0=psA[:], scalar=1.0, in1=bS[:],
                    op0=Alu.subtract, op1=Alu.mult,
                    accum_out=partials[:, t:t + 1])

            for t in range(T):
                s4 = ipool.tile([P, 4, TILE_E], f8, tag="s4")
                in_ap = bass.AP(tensor=in4[:].tensor,
                                offset=t * 4 * TILE_E,
                                ap=[[T * 4 * TILE_E, P], [TILE_E, 4],
                                    [1, TILE_E]])
                nc.sync.dma_start(out=s4[:], in_=in_ap)

                w_r = wpool.tile([P, TILE_E], bf16, tag="w_r")
                diffV = wpool.tile([P, TILE_E - PECOL], bf16, tag="diffV")
                sq = wpool.tile([P, TILE_E], bf16, tag="sq")

                nc.vector.tensor_tensor(out=w_r[:], in0=s4[:, 0, :],
                                        in1=s4[:, 1, :], op=Alu.mult)
                nc.vector.tensor_tensor(out=diffV[:],
                                        in0=s4[:, 2, PECOL:],
                                        in1=s4[:, 3, PECOL:],
                                        op=Alu.subtract)

                # one 512-col PSUM bank = one accumulation region; a
                # single adjacent (I @ u, then -I @ v accumulate) pair
                # per bank avoids intra-bank group interleaving, which
                # corrupts PSUM accumulation.
                for b in range(NBAT):
                    psD = pdpool.tile([P, BB * P], fp32, tag="psD")
                    c0 = b * BB * P
                    nc.tensor.matmul(psD[:], eyeP[:],
                                     s4[:, 2, c0:c0 + BB * P],
                                     start=True, stop=False)
                    nc.tensor.matmul(psD[:], eyeN[:],
                                     s4[:, 3, c0:c0 + BB * P],
                                     start=False, stop=True)
                    nc.scalar.activation(out=sq[:, c0:c0 + BB * P],
                                         in_=psD[:], func=Sq)
                nc.scalar.activation(out=sq[:, PECOL:], in_=diffV[:],
                                     func=Sq)

                pend_red[t] = (w_r, sq)
                if t >= 1:
                    reduce_stage(t - 1)
                if t >= LAG:
                    combine(t - LAG)

            reduce_stage(T - 1)
            for t in range(max(0, T - LAG), T):
                combine(t)

            total = stpool.tile([P, 1], fp32, tag="total")
            nc.vector.tensor_reduce(out=total[:], in_=partials[:], axis=X,
                                    op=Alu.add)
            nc.sync.dma_start(out=out[:], in_=total[:])
    nc.compile()
    _cache["nc"] = nc
    return nc


def kernel(re_, ir_h, src, dst):
    re_ = np.asarray(re_, dtype=np.float32)
    ir_h = np.asarray(ir_h, dtype=np.float32)
    g2r = np.ascontiguousarray(
        re_.T.astype(ml_dtypes.float8_e4m3))        # [128, N]
    g2h = np.ascontiguousarray(
        ir_h.T.astype(ml_dtypes.float8_e4m3))       # [128, N]
    eye = np.concatenate([np.eye(P, dtype=np.float32),
                          -np.eye(P, dtype=np.float32)]
                         ).astype(ml_dtypes.float8_e4m3)

    s = np.asarray(src).astype(np.int64)
    d = np.asarray(dst).astype(np.int64)
    e_total = s.shape[0]

    in_maps = []
    for c in range(N_CORES):
        lo, hi = c * EPC, (c + 1) * EPC
        arr = np.zeros((4, P, PAD_E), ml_dtypes.float8_e4m3)
        arr[0, :, :EPC] = g2r[:, s[lo:hi]]
        arr[1, :, :EPC] = g2r[:, d[lo:hi]]
        arr[2, :, :EPC] = g2h[:, s[lo:hi]]
        arr[3, :, :EPC] = g2h[:, d[lo:hi]]
        # [4, P, T, TILE_E] -> [P, T, 4, TILE_E] (16 KB contiguous per
        # partition per tile)
        a4 = np.ascontiguousarray(
            arr.reshape(4, P, T, TILE_E).transpose(1, 2, 0, 3))
        in_maps.append({"in4": a4.reshape(P, T * 4 * TILE_E), "eye": eye})

    nc = _build_program()
    try:
        res = run_bass_kernel_spmd(nc, in_maps,
                                   core_ids=list(range(N_CORES)))
    except Exception:
        # transient NRT device wedge -- one retry is usually enough
        res = run_bass_kernel_spmd(nc, in_maps,
                                   core_ids=list(range(N_CORES)))
    tot = 0.0
    for r in res.results:
        tot += float(r["partial"].sum(dtype=np.float64))
    return np.float32(-tot / e_total)


# revision 35
# speedup vs baseline: 1.3392x; 1.0174x over previous
"""Trainium2 Bass kernel for nn_Ir_Consistency_Loss (gnn_message_passing).

loss = mean_e (1 - re[src_e].re[dst_e]) * ||ir_h[src_e] - ir_h[dst_e]||^2

Pure-streaming, edge-parallel design across 8 NeuronCores, transposed
(feature-dim-on-partitions) layout, fp8_e4m3 stream dtype.

The host pre-gathers per-edge node rows for BOTH endpoints into one
fp8 tensor per core (feature dim on partitions), laid out
[d, tile, j, e] so each partition reads 16 KB contiguous per tile:

  j = 0:re[src] 1:re[dst] 2:ir[src] 3:ir[dst]

Engine facts measured on HW:
  - DVE tensor_tensor runs 1x on fp8 (~1.04 ns/elem/lane).
  - Pool (gpsimd) elementwise shares SBUF ports with DVE; offloading
    there is a net LOSS. Pool does nothing here.
  - PE can compute u - v into PSUM via two accumulated identity
    matmuls (lhsT=I then lhsT=-I), offloading elementwise subtracts.
  - ACT activation is 1x, dtype-independent, (N+352)/1.2 ns, and can
    read PSUM directly.

Per 4096-edge tile (one 2.1 MB DMA):
  - DVE:  w_r = u_r * v_r (fp8->bf16, all 4096 cols)
          diffV = u_h - v_h on cols [PECOL:4096]
  - PE :  diff on cols [0:PECOL) as 512-col PSUM banks:
          psD = I @ u_cols - I @ v_cols  (fp8 identity weights; one
          bank = one accumulation region, pairs kept adjacent)
  - ACT:  sq[:, batch] = Square(psD) per batch; sq[:, 2560:] =
          Square(diffV); all into one sq tile
  - PE :  (emitted next iteration to avoid in-order stalls) per
          128-edge chunk c, ones-matmuls reduce over the partition
          axis: psA[:,c] = agree_e, psB[:,c] = sqsum_e
  - ACT:  stages psB into SBUF (DVE has a single PSUM read port)
  - DVE:  scalar_tensor_tensor (agree-1)*sqsum -> per-tile partials
          (emitted 2 iterations late, same reason)
  - Pad edges are all-zero rows: (0 - 1) * 0 = 0 contribution.
  - Host: loss = -(sum of per-core partials) / E.
"""

import numpy as np
import ml_dtypes

import concourse.bacc as bacc
import concourse.bass as bass
import concourse.mybir as mybir
import concourse.tile as tile
from concourse.bass_utils import run_bass_kernel_spmd

N_NODES = 50000
N_EDGES = 1600000
D = 128
N_CORES = 8
P = 128
CHUNK = 32                 # 128-edge chunks per tile
TILE_E = P * CHUNK         # 4096 edges per tile
EPC = N_EDGES // N_CORES   # 200000 edges per core
T = -(-EPC // TILE_E)      # 49 tiles per core
PAD_E = T * TILE_E         # 200704 padded edges per core
PEC = 24                   # diff chunks computed on PE (cols [0:PEC*128))
BB = 4                     # chunks per PSUM diff batch (512 cols = 1 bank)
NBAT = PEC // BB           # PSUM diff batches per tile
PECOL = PEC * P            # 2560

_cache = {}


def _build_program():
    if "nc" in _cache:
        return _cache["nc"]
    nc = bacc.Bacc("TRN2", target_bir_lowering=False, debug=False,
                   num_devices=N_CORES)
    f8 = mybir.dt.float8e4
    bf16 = mybir.dt.bfloat16
    fp32 = mybir.dt.float32
    # [P, T, 4, TILE_E]: each partition reads 16 KB contiguous per tile
    in4 = nc.dram_tensor("in4", [P, T * 4 * TILE_E], f8,
                         kind="ExternalInput")
    eye = nc.dram_tensor("eye", [2 * P, P], f8, kind="ExternalInput")
    out = nc.dram_tensor("partial", [P, 1], fp32, kind="ExternalOutput")

    Alu = mybir.AluOpType
    X = mybir.AxisListType.X
    Sq = mybir.ActivationFunctionType.Square
    Cp = mybir.ActivationFunctionType.Copy

    LAG = 2  # iterations of lag for the combine stage

    with tile.TileContext(nc) as tc:
        with (
            tc.tile_pool(name="in", bufs=5) as ipool,
            tc.tile_pool(name="work", bufs=4) as wpool,
            tc.tile_pool(name="psd", bufs=3, space="PSUM") as pdpool,
            tc.tile_pool(name="ps", bufs=3, space="PSUM") as pspool,
            tc.tile_pool(name="cmb", bufs=LAG + 2) as cpool,
            tc.tile_pool(name="stats", bufs=1) as stpool,
        ):
            partials = stpool.tile([P, T], fp32, tag="partials")
            ones = stpool.tile([P, 1], bf16, tag="ones")
            nc.vector.memset(ones[:], 1.0)
            eyeP = stpool.tile([P, P], f8, tag="eyeP")
            eyeN = stpool.tile([P, P], f8, tag="eyeN")
            nc.sync.dma_start(out=eyeP[:], in_=eye[0:P, :])
            nc.sync.dma_start(out=eyeN[:], in_=eye[P:2 * P, :])

            pend_red = {}
            pend_cmb = {}

            def reduce_stage(t):
                # ones-matmul reductions for tile t (emitted during
                # iteration t+1 so PE never waits on ACT mid-queue)
                w_r, sq = pend_red.pop(t)
                psAB = pspool.tile([P, 2, CHUNK], fp32, tag="psAB")
                psA = psAB[:, 0, :]
                psB = psAB[:, 1, :]
                for c in range(CHUNK):
                    nc.tensor.matmul(psA[:, c:c + 1],
                                     w_r[:, c * P:(c + 1) * P], ones[:],
                                     start=True, stop=True)
                    nc.tensor.matmul(psB[:, c:c + 1],
                                     sq[:, c * P:(c + 1) * P], ones[:],
                                     start=True, stop=True)
                pend_cmb[t] = psAB

            def combine(t):
                psAB = pend_cmb.pop(t)
                psA = psAB[:, 0, :]
                psB = psAB[:, 1, :]
                bS = cpool.tile([P, CHUNK], fp32, tag="bS")
                junk = cpool.tile([P, CHUNK], fp32, tag="junk")
                # DVE may read at most one PSUM operand per instruction
                # (single PSUM read port) -> ACT stages psB into SBUF.
                nc.scalar.activation(out=bS[:], in_=psB[:], func=Cp)
                nc.vector.scalar_tensor_tensor(
                    out=junk[:], in0=psA[:], scalar=1.0, in1=bS[:],
                    op0=Alu.subtract, op1=Alu.mult,
                    accum_out=partials[:, t:t + 1])

            for t in range(T):
                s4 = ipool.tile([P, 4, TILE_E], f8, tag="s4")
                in_ap = bass.AP(tensor=in4[:].tensor,
                                offset=t * 4 * TILE_E,
                                ap=[[T * 4 * TILE_E, P], [TILE_E, 4],
                                    [1, TILE_E]])
                nc.sync.dma_start(out=s4[:], in_=in_ap)

                w_r = wpool.tile([P, TILE_E], bf16, tag="w_r")
                diffV = wpool.tile([P, TILE_E - PECOL], bf16, tag="diffV")
                sq = wpool.tile([P, TILE_E], bf16, tag="sq")

                nc.vector.tensor_tensor(out=w_r[:], in0=s4[:, 0, :],
                                        in1=s4[:, 1, :], op=Alu.mult)
                nc.vector.tensor_tensor(out=diffV[:],
                                        in0=s4[:, 2, PECOL:],
                                        in1=s4[:, 3, PECOL:],
                                        op=Alu.subtract)

                # one 512-col PSUM bank = one accumulation region; a
                # single adjacent (I @ u, then -I @ v accumulate) pair
                # per bank avoids intra-bank group interleaving, which
                # corrupts PSUM accumulation.
                for b in range(NBAT):
                    psD = pdpool.tile([P, BB * P], fp32, tag="psD")
                    c0 = b * BB * P
                    nc.tensor.matmul(psD[:], eyeP[:],
                                     s4[:, 2, c0:c0 + BB * P],
                                     start=True, stop=False)
                    nc.tensor.matmul(psD[:], eyeN[:],
                                     s4[:, 3, c0:c0 + BB * P],
                                     start=False, stop=True)
                    nc.scalar.activation(out=sq[:, c0:c0 + BB * P],
                                         in_=psD[:], func=Sq)
                nc.scalar.activation(out=sq[:, PECOL:], in_=diffV[:],
                                     func=Sq)

                pend_red[t] = (w_r, sq)
                if t >= 1:
                    reduce_stage(t - 1)
                if t >= LAG:
                    combine(t - LAG)

            reduce_stage(T - 1)
            for t in range(max(0, T - LAG), T):
                combine(t)

            total = stpool.tile([P, 1], fp32, tag="total")
            nc.vector.tensor_reduce(out=total[:], in_=partials[:], axis=X,
                                    op=Alu.add)
            nc.sync.dma_start(out=out[:], in_=total[:])
    nc.compile()
    _cache["nc"] = nc
    return nc


def kernel(re_, ir_h, src, dst):
    re_ = np.asarray(re_, dtype=np.float32)
    ir_h = np.asarray(ir_h, dtype=np.float32)
    g2r = np.ascontiguousarray(
        re_.T.astype(ml_dtypes.float8_e4m3))        # [128, N]
    g2h = np.ascontiguousarray(
        ir_h.T.astype(ml_dtypes.float8_e4m3))       # [128, N]
    eye = np.concatenate([np.eye(P, dtype=np.float32),
                          -np.eye(P, dtype=np.float32)]
                         ).astype(ml_dtypes.float8_e4m3)

    s = np.asarray(src).astype(np.int64)
    d = np.asarray(dst).astype(np.int64)
    e_total = s.shape[0]

    in_maps = []
    for c in range(N_CORES):
        lo, hi = c * EPC, (c + 1) * EPC
        arr = np.zeros((4, P, PAD_E), ml_dtypes.float8_e4m3)
        arr[0, :, :EPC] = g2r[:, s[lo:hi]]
        arr[1, :, :EPC] = g2r[:, d[lo:hi]]
        arr[2, :, :EPC] = g2h[:, s[lo:hi]]
        arr[3, :, :EPC] = g2h[:, d[lo:hi]]
        # [4, P, T, TILE_E] -> [P, T, 4, TILE_E] (16 KB contiguous per
        # partition per tile)
        a4 = np.ascontiguousarray(
            arr.reshape(4, P, T, TILE_E).transpose(1, 2, 0, 3))
        in_maps.append({"in4": a4.reshape(P, T * 4 * TILE_E), "eye": eye})

    nc = _build_program()
    try:
        res = run_bass_kernel_spmd(nc, in_maps,
                                   core_ids=list(range(N_CORES)))
    except Exception:
        # transient NRT device wedge -- one retry is usually enough
        res = run_bass_kernel_spmd(nc, in_maps,
                                   core_ids=list(range(N_CORES)))
    tot = 0.0
    for r in res.results:
        tot += float(r["partial"].sum(dtype=np.float64))
    return np.float32(-tot / e_total)
